# revision 7
# baseline (speedup 1.0000x reference)
"""BatchAlignmentLoss on 8 Trainium2 NeuronCores.

Strategy: shard the feature dim D=2048 across 8 cores (256 cols each).
Each core streams its [8192, 256] slice of the three feature matrices,
computes partial row sq-norms (AllReduce'd per 2048-row chunk, 24 KiB),
and segment-reduces rows into [512, 256] class sums on the PE using
diag(1/n_row) as the stationary operand (labels follow the arange%512
pattern, so each 128-row tile maps 1:1 onto a 128-class tile; a general
one-hot path covers arbitrary labels).  The tail computes center norms
(6 KiB AllReduce), pairwise logits partials (contraction over the local
256 cols), ReduceScatters the [512,512]x3 logits so each core
log-softmaxes 64 rows per pair, and a final [128,8] AllReduce combines
the intra/inter partial sums into the scalar loss.

Algebraic simplifications vs the reference (exact up to fp rounding):
  l2norm(s * inv_cnt) == l2norm(s)           (counts cancel)
  mean||f - c[label]||^2 == 2 - (2/N) * sum_p s_p . c_p
so neither counts nor a per-sample center gather are needed.
"""

import numpy as np

N = 8192
P = 512
D = 2048
NCORES = 8
DL = D // NCORES          # 256 cols per core
TAU = 0.5
NJ = 16                   # N // P occurrences per class (fast path)
NCHUNK = 4                # phase-A chunks (2048 rows each)

_CACHE = {}


def _legalize_waits(nc, mybir):
    """This walrus build accepts at most 1 sync wait per instruction
    (2 on InstEventSemaphore); Tile's scheduler can attach more. Hoist
    the extras onto fresh single-wait nops inserted just before the
    offending instruction (same engine, so ordering is preserved)."""
    for fn in nc.m.functions:
        for bb in fn.blocks:
            insts = bb.instructions
            i = 0
            while i < len(insts):
                inst = insts[i]
                si = getattr(inst, "sync_info", None)
                if si is None:
                    i += 1
                    continue
                waits = list(si.on_wait)
                cap = 2 if isinstance(inst, mybir.InstEventSemaphore) else 1
                if len(waits) <= cap:
                    i += 1
                    continue
                extras, keep = waits[:-cap], waits[-cap:]
                inst.sync_info = mybir.SyncInfo(
                    on_wait=keep, on_update=list(si.on_update))
                for k, w in enumerate(extras):
                    nop = mybir.InstNoOp(
                        name=f"{inst.name}.w{k}",
                        sync_info=mybir.SyncInfo(on_wait=[w], on_update=[]),
                        bass_nofuse=True,
                        engine=inst.engine,
                    )
                    nc.register_instruction(nop, overwrite=True)
                    insts.insert(i, nop)
                    i += 1
                i += 1


def _build_program(fast):
    from concourse import bass, mybir
    from concourse import tile as tile_mod

    f32 = mybir.dt.float32
    Alu = mybir.AluOpType
    Act = mybir.ActivationFunctionType
    Ax = mybir.AxisListType

    nc = bass.Bass()
    fin = {}
    for name in ("fv", "fa", "fr"):
        fin[name] = nc.declare_dram_parameter(name, [N, DL], f32, isOutput=False)
    fmats = [fin["fv"], fin["fa"], fin["fr"]]
    dcol_ext = nc.declare_dram_parameter("dcol", [64, 1], f32, isOutput=False)
    if not fast:
        labm_ext = nc.declare_dram_parameter("labm", [128, 64], f32, isOutput=False)
    loss_ext = nc.declare_dram_parameter("loss", [1, 1], f32, isOutput=True)

    rg = [list(range(NCORES))]

    with tile_mod.TileContext(nc) as tc:
        with (
            tc.tile_pool(name="sb", bufs=2) as sb,
            tc.tile_pool(name="sb1", bufs=1) as sb1,
            tc.tile_pool(name="dram", bufs=2, space="DRAM") as dram,
        ):
            # ---- constants / setup ----
            ones128 = sb1.tile([128, 128], f32, tag="ones128")
            nc.vector.memset(ones128[:], 1.0)
            ident = sb1.tile([128, 128], f32, tag="ident")
            nc.gpsimd.affine_select(
                ident[:], ones128[:], pattern=[[-1, 128]], base=0,
                channel_multiplier=1, compare_op=Alu.is_equal, fill=0.0,
            )
            dcol = sb1.tile([64, 1], f32, tag="dcol")
            nc.sync.dma_start(dcol[:], dcol_ext[:])
            iota512 = sb1.tile([64, 512], f32, tag="iota512")
            nc.gpsimd.iota(iota512[:], pattern=[[1, 512]], base=0,
                           channel_multiplier=0,
                           allow_small_or_imprecise_dtypes=True)
            dgmask = sb1.tile([64, 512], f32, tag="dgmask")
            nc.vector.tensor_scalar(dgmask[:], iota512[:], dcol[:], None,
                                    Alu.is_equal)
            wvec = sb1.tile([1, 8], f32, tag="wvec")
            nc.vector.memset(wvec[:, 0:3], -2.0 / N)
            nc.vector.memset(wvec[:, 3:6], -1.0 / P)
            nc.vector.memset(wvec[:, 6:8], 0.0)
            if not fast:
                labm = sb1.tile([128, 64], f32, tag="labm")
                nc.sync.dma_start(labm[:], labm_ext[:])
                iota128 = sb1.tile([128, 128], f32, tag="iota128")
                nc.gpsimd.iota(iota128[:], pattern=[[1, 128]], base=0,
                               channel_multiplier=0,
                               allow_small_or_imprecise_dtypes=True)

            # ---- phase A: stream + row norms + segment matmuls ----
            # PSUM: one accumulation group per bank. v|a fused as a
            # [128,512] rhs into 4 full banks; r alone in 4 half-banks.
            with tc.tile_pool(name="ps_s", bufs=1, space="PSUM") as ps_s:
                s_va = [ps_s.tile([128, 512], f32, name=f"sva{q}", tag=f"sva{q}")
                        for q in range(4)]
                s_r = [ps_s.tile([128, 256], f32, name=f"sr{q}", tag=f"sr{q}")
                       for q in range(4)]

                for c in range(NCHUNK):
                    # -- load: 6 x 1 MiB DMAs into two [128,*,768] tiles --
                    ftiles = {}
                    for h in range(2):
                        if fast:
                            t = sb.tile([128, 2, 4, 768], f32, tag=f"f{h}")
                        else:
                            t = sb.tile([128, 8, 1, 768], f32, tag=f"f{h}")
                        for m in range(3):
                            if fast:
                                r0 = 512 * (4 * c + 2 * h)
                                src_ap = fmats[m][r0:r0 + 1024, :].rearrange(
                                    "(j ct p) d -> p j ct d", j=2, ct=4, p=128)
                            else:
                                r0 = 128 * (16 * c + 8 * h)
                                src_ap = fmats[m][r0:r0 + 1024, :].rearrange(
                                    "(x p) d -> p x () d", x=8, p=128)
                            nc.sync.dma_start(
                                t[:, :, :, 256 * m:256 * m + 256], src_ap)
                        ftiles[h] = t

                    # slice list: (h, inner, m) -> [128, 256] view + pack col
                    def views():
                        for h in range(2):
                            t = ftiles[h]
                            if fast:
                                for j2 in range(2):
                                    for ct in range(4):
                                        for m in range(3):
                                            jj = 2 * h + j2
                                            col = (jj * 3 + m) * 4 + ct
                                            yield t[:, j2, ct,
                                                    256 * m:256 * m + 256], col
                            else:
                                for x in range(8):
                                    for m in range(3):
                                        xx = 8 * h + x
                                        col = xx * 3 + m
                                        yield t[:, x, 0,
                                                256 * m:256 * m + 256], col

                    # -- row sq-norm partials -> sqpack [128, 48] --
                    sqpack = sb.tile([128, 48], f32, tag="sqpack")
                    for i, (v, col) in enumerate(views()):
                        acc = sqpack[:, col:col + 1]
                        if i % 3 != 2:
                            scr = sb.tile([128, 256], f32, tag="scrA")
                            nc.scalar.activation(scr[:], v, Act.Square,
                                                 accum_out=acc)
                        else:
                            scr = sb.tile([128, 256], f32, tag="scrV")
                            nc.vector.tensor_tensor(scr[:], v, v, Alu.mult)
                            nc.vector.tensor_reduce(acc, scr[:], Ax.X, Alu.add)

                    # -- AllReduce the 24 KiB of partial sq-norms --
                    nin = dram.tile([128, 48], f32, tag="nin")
                    nout = dram.tile([128, 48], f32, tag="nout")
                    nc.sync.dma_start(nin[:], sqpack[:])
                    nc.gpsimd.collective_compute(
                        "AllReduce", Alu.add, replica_groups=rg,
                        ins=[nin.opt()], outs=[nout.opt()])
                    sqg = sb.tile([128, 48], f32, tag="sqg")
                    nc.sync.dma_start(sqg[:], nout[:])

                    # -- 1 / max(sqrt(q), eps) --
                    nsr = sb.tile([128, 48], f32, tag="nsr")
                    nc.scalar.activation(nsr[:], sqg[:], Act.Sqrt)
                    nmx = sb.tile([128, 48], f32, tag="nmx")
                    nc.vector.tensor_scalar(nmx[:], nsr[:], 1e-12, None, Alu.max)
                    rinv = sb.tile([128, 48], f32, tag="rinv")
                    nc.vector.reciprocal(rinv[:], nmx[:])

                    # -- scale rows in place by 1/n (2 of 3 on DVE) --
                    for i, (v, col) in enumerate(views()):
                        r_ap = rinv[:, col:col + 1]
                        if i % 3 == 0:
                            nc.scalar.activation(v, v, Act.Copy, scale=r_ap)
                        else:
                            nc.vector.tensor_scalar(v, v, r_ap, None, Alu.mult)

                    # -- segment accumulate on PE (identity lhsT) --
                    for h in range(2):
                        t = ftiles[h]
                        if fast:
                            for j2 in range(2):
                                j = 4 * c + 2 * h + j2
                                for ct in range(4):
                                    nc.tensor.matmul(
                                        s_va[ct][:], ident[:],
                                        t[:, j2, ct, 0:512],
                                        start=(j == 0), stop=(j == NJ - 1))
                                    nc.tensor.matmul(
                                        s_r[ct][:], ident[:],
                                        t[:, j2, ct, 512:768],
                                        start=(j == 0), stop=(j == NJ - 1))
                        else:
                            for x in range(8):
                                rt = 16 * c + 8 * h + x
                                for ps in range(4):
                                    oh = sb.tile([128, 128], f32, tag="oh")
                                    nc.vector.tensor_scalar(
                                        oh[:], iota128[:], labm[:, rt:rt + 1],
                                        float(-128 * ps), Alu.subtract,
                                        Alu.is_equal)
                                    nc.tensor.matmul(
                                        s_va[ps][:], oh[:], t[:, x, 0, 0:512],
                                        start=(rt == 0), stop=(rt == 63))
                                    nc.tensor.matmul(
                                        s_r[ps][:], oh[:], t[:, x, 0, 512:768],
                                        start=(rt == 0), stop=(rt == 63))

                # -- evacuate segment sums PSUM -> SBUF --
                s_sb = [[sb1.tile([128, 512], f32, name=f"ssb{m}{h}",
                                  tag=f"ssb{m}{h}")
                         for h in range(2)] for m in range(3)]
                for q in range(4):
                    dst = [(0, s_va[q][:, 0:256]), (1, s_va[q][:, 256:512]),
                           (2, s_r[q][:])]
                    for m, src_ap in dst:
                        nc.vector.tensor_copy(
                            s_sb[m][q // 2][:, 256 * (q % 2):256 * (q % 2) + 256],
                            src_ap)

            def sb_slice(mat, q):
                return mat[q // 2][:, 256 * (q % 2):256 * (q % 2) + 256]

            # ---- tail ----
            with tc.tile_pool(name="ps_t", bufs=2, space="PSUM") as ps_t, \
                 tc.tile_pool(name="ps_l", bufs=1, space="PSUM") as ps_l, \
                 tc.tile_pool(name="ps_f", bufs=1, space="PSUM") as ps_f:

                # center sq-norm partials [128, 12] (col = 4*m + q)
                qpack = sb1.tile([128, 12], f32, tag="qpack")
                for m in range(3):
                    for q in range(4):
                        scr = sb.tile([128, 256], f32, tag="scrA")
                        nc.scalar.activation(
                            scr[:], sb_slice(s_sb[m], q), Act.Square,
                            accum_out=qpack[:, 4 * m + q:4 * m + q + 1])
                qin = dram.tile([128, 12], f32, tag="qin")
                qout = dram.tile([128, 12], f32, tag="qout")
                nc.sync.dma_start(qin[:], qpack[:])
                nc.gpsimd.collective_compute(
                    "AllReduce", Alu.add, replica_groups=rg,
                    ins=[qin.opt()], outs=[qout.opt()])
                qg = sb1.tile([128, 12], f32, tag="qg")
                nc.sync.dma_start(qg[:], qout[:])

                csqrt = sb1.tile([128, 12], f32, tag="csqrt")
                nc.scalar.activation(csqrt[:], qg[:], Act.Sqrt)
                cmx = sb1.tile([128, 12], f32, tag="cmx")
                nc.vector.tensor_scalar(cmx[:], csqrt[:], 1e-11, None, Alu.max)
                rc = sb1.tile([128, 12], f32, tag="rc")
                nc.vector.reciprocal(rc[:], cmx[:])

                # final pack: cols 0-2 intra dots, 3-5 inter sums
                finpack = sb1.tile([128, 8], f32, tag="finpack")
                nc.vector.memset(finpack[:], 0.0)
                for m in range(3):
                    scr4 = sb.tile([128, 4], f32, tag="scr4")
                    nc.vector.tensor_tensor(
                        scr4[:], qpack[:, 4 * m:4 * m + 4],
                        rc[:, 4 * m:4 * m + 4], Alu.mult)
                    nc.vector.tensor_reduce(
                        finpack[:, m:m + 1], scr4[:], Ax.X, Alu.add)

                # centers (scaled in a copy), then transpose to [d, p] layout
                c_sb = [[sb1.tile([128, 512], f32, name=f"csb{m}{h}", tag=f"csb{m}{h}")
                         for h in range(2)] for m in range(3)]
                for m in range(3):
                    for q in range(4):
                        nc.vector.tensor_scalar(
                            sb_slice(c_sb[m], q), sb_slice(s_sb[m], q),
                            rc[:, 4 * m + q:4 * m + q + 1], None, Alu.mult)
                cT = [sb1.tile([128, 2, 512], f32, name=f"cT{m}", tag=f"cT{m}") for m in range(3)]
                for m in range(3):
                    for q in range(4):
                        for kd in range(2):
                            tp = ps_t.tile([128, 128], f32, tag="tp")
                            blk = c_sb[m][q // 2][:, 256 * (q % 2) + 128 * kd:
                                                  256 * (q % 2) + 128 * kd + 128]
                            nc.tensor.transpose(tp[:], blk, ident[:])
                            nc.vector.tensor_copy(
                                cT[m][:, kd, 128 * q:128 * q + 128], tp[:])

                # pairwise logits partials, scaled by 1/TAU, into RS bounce
                rs_in = dram.tile([NCORES, 3, 64, 512], f32, tag="rs_in")
                rs_out = dram.tile([3, 64, 512], f32, tag="rs_out")
                pairs = [(0, 1), (0, 2), (1, 2)]
                for pi, (A, B) in enumerate(pairs):
                    for pt in range(4):
                        lg = ps_l.tile([128, 512], f32, tag=f"lg{pt}")
                        for kd in range(2):
                            nc.tensor.matmul(
                                lg[:], cT[A][:, kd, 128 * pt:128 * pt + 128],
                                cT[B][:, kd, :], start=(kd == 0), stop=(kd == 1))
                        lgs = sb.tile([128, 512], f32, tag="lgs")
                        nc.vector.tensor_scalar(lgs[:], lg[:], 1.0 / TAU, None,
                                                Alu.mult)
                        nc.sync.dma_start(rs_in[2 * pt:2 * pt + 2, pi, :, :], lgs[:])
                nc.gpsimd.collective_compute(
                    "ReduceScatter", Alu.add, replica_groups=rg,
                    ins=[rs_in.opt()], outs=[rs_out.opt()])
                lgl = sb1.tile([64, 3, 512], f32, tag="lgl")
                nc.sync.dma_start(lgl[:], rs_out[:].rearrange("pi p q -> p pi q"))

                # row log-softmax diag on this core's 64 rows of each pair
                for pi in range(3):
                    row = lgl[:, pi, :]
                    mxn = sb.tile([64, 1], f32, tag="mxn")
                    nc.vector.tensor_reduce(mxn[:], row, Ax.X, Alu.max,
                                            negate=True)
                    escr = sb.tile([64, 512], f32, tag="escr")
                    se = sb.tile([64, 1], f32, tag="se")
                    nc.scalar.activation(escr[:], row, Act.Exp, bias=mxn[:],
                                         accum_out=se[:])
                    lse = sb.tile([64, 1], f32, tag="lse")
                    nc.scalar.activation(lse[:], se[:], Act.Ln)
                    dscr = sb.tile([64, 512], f32, tag="dscr")
                    dg = sb.tile([64, 1], f32, tag="dgv")
                    nc.vector.tensor_tensor(dscr[:], row, dgmask[:], Alu.mult)
                    nc.vector.tensor_reduce(dg[:], dscr[:], Ax.X, Alu.add)
                    t1 = sb.tile([64, 1], f32, tag="t1")
                    nc.vector.tensor_tensor(t1[:], dg[:], mxn[:], Alu.add)
                    nc.vector.tensor_tensor(
                        finpack[0:64, 3 + pi:4 + pi], t1[:], lse[:], Alu.subtract)

                # final AllReduce + partition sum + weighted combine
                fin_i = dram.tile([128, 8], f32, tag="fin_i")
                fin_o = dram.tile([128, 8], f32, tag="fin_o")
                nc.sync.dma_start(fin_i[:], finpack[:])
                nc.gpsimd.collective_compute(
                    "AllReduce", Alu.add, replica_groups=rg,
                    ins=[fin_i.opt()], outs=[fin_o.opt()])
                fing = sb1.tile([128, 8], f32, tag="fing")
                nc.sync.dma_start(fing[:], fin_o[:])
                csum = ps_f.tile([1, 8], f32, tag="csum")
                nc.tensor.matmul(csum[:], ones128[:, 0:1], fing[:],
                                 start=True, stop=True)
                fsum = sb1.tile([1, 8], f32, tag="fsum")
                nc.vector.tensor_copy(fsum[:], csum[:])
                scr8 = sb1.tile([1, 8], f32, tag="scr8")
                lsum = sb1.tile([1, 1], f32, tag="lsum")
                loss = sb1.tile([1, 1], f32, tag="loss")
                nc.vector.tensor_tensor(scr8[:], fsum[:], wvec[:], Alu.mult)
                nc.vector.tensor_reduce(lsum[:], scr8[:], Ax.X, Alu.add)
                nc.vector.tensor_scalar(loss[:], lsum[:], 6.0, None, Alu.add)
                nc.sync.dma_start(loss_ext[:], loss[:])

    _legalize_waits(nc, mybir)
    return nc


def _get_program(fast):
    key = ("prog", fast)
    if key not in _CACHE:
        _CACHE[key] = _build_program(fast)
    return _CACHE[key]


def _make_in_maps(feat_vp, feat_ap, feat_rp, label, fast):
    in_maps = []
    for k in range(NCORES):
        m = {
            "fv": np.ascontiguousarray(feat_vp[:, DL * k:DL * (k + 1)]),
            "fa": np.ascontiguousarray(feat_ap[:, DL * k:DL * (k + 1)]),
            "fr": np.ascontiguousarray(feat_rp[:, DL * k:DL * (k + 1)]),
            "dcol": np.arange(64 * k, 64 * k + 64, dtype=np.float32).reshape(64, 1),
        }
        if not fast:
            m["labm"] = np.ascontiguousarray(
                label.astype(np.float32).reshape(64, 128).T)
        in_maps.append(m)
    return in_maps


def kernel(feat_vp, feat_ap, feat_rp, label, _trace=False):
    from concourse.bass_utils import run_bass_kernel_spmd

    feat_vp = np.asarray(feat_vp, dtype=np.float32)
    feat_ap = np.asarray(feat_ap, dtype=np.float32)
    feat_rp = np.asarray(feat_rp, dtype=np.float32)
    label = np.asarray(label)
    fast = bool((label == (np.arange(N) % P).astype(label.dtype)).all())

    nc = _get_program(fast)
    in_maps = _make_in_maps(feat_vp, feat_ap, feat_rp, label, fast)
    res = run_bass_kernel_spmd(nc, in_maps, list(range(NCORES)), trace=_trace)
    out = np.asarray(res.results[0]["loss"], dtype=np.float32).reshape(())
    if _trace:
        return out, res
    return out
